# revision 4
# baseline (speedup 1.0000x reference)
"""Sharded attention-energy + softmax kernel for 8 trn2 NeuronCores.

Math: energies = (E @ W.T + b) @ hidden = E @ (hidden @ W) + (b.hidden)
The (b.hidden) term is a constant shift of all logits, which softmax
cancels exactly, so the device only computes e = E @ u with
u = hidden @ W (tiny host-side matvec), then a numerically-stable
sharded softmax: each core emits exp(e - rowmax) plus per-partition
max/sum stats; the host performs the standard two-pass softmax merge.

Sharding: encoder_outputs [32768, 1024] split along seq into 8 shards
of [4096, 1024] (one per core); u replicated.
"""

import numpy as np

H = 1024
S = 32768
NCORES = 8
SSH = S // NCORES          # 4096 seq rows per core
P = 128                    # SBUF partitions
Q = 4                      # row-groups per supertile
NSUP = SSH // (P * Q)      # 8 supertiles per core
NCOL = SSH // P            # 32 energy columns per core
LOAD_BUFS = 4

_nc = None


def _build():
    import concourse.bass as bass
    import concourse.bacc as bacc
    import concourse.tile as tile
    from concourse import mybir

    f32 = mybir.dt.float32
    nc = bacc.Bacc()

    enc = nc.declare_dram_parameter("enc", [SSH, H], f32, isOutput=False)
    u = nc.declare_dram_parameter("u", [H], f32, isOutput=False)
    # out[:, :NCOL] = exp(e - m) ; out[:, NCOL] = m ; out[:, NCOL+1] = sum
    out = nc.declare_dram_parameter("out", [P, NCOL + 2], f32, isOutput=True)

    # row s = n*Q*P + q*P + p  ->  supertile n, free-block q, partition p
    enc_v = enc[:].rearrange("(n q p) h -> n p q h", q=Q, p=P)

    with tile.TileContext(nc) as tc:
        with (
            tc.tile_pool(name="singles", bufs=1) as singles,
            tc.tile_pool(name="loads", bufs=LOAD_BUFS) as loads,
        ):
            u_b = singles.tile([P, H], f32)
            nc.gpsimd.dma_start(out=u_b, in_=u[None, :].to_broadcast([P, H]))

            e_sb = singles.tile([P, NCOL], f32)
            dummy = singles.tile([P, 1], f32)

            for n in range(NSUP):
                t = loads.tile([P, Q, H], f32)
                nc.sync.dma_start(out=t, in_=enc_v[n])
                for q in range(Q):
                    col = n * Q + q
                    nc.vector.affine_mul_reduce(
                        out=dummy.broadcast_to([P, H]),
                        accum_out=e_sb[:, col : col + 1],
                        in0=t[:, q, :],
                        in1=u_b,
                        scale=1.0,
                        bias=0.0,
                    )

            combo = singles.tile([P, NCOL + 2], f32)
            neg = singles.tile([P, 1], f32)
            nc.vector.tensor_reduce(
                out=combo[:, NCOL : NCOL + 1],
                in_=e_sb,
                axis=mybir.AxisListType.X,
                op=mybir.AluOpType.max,
            )
            nc.vector.tensor_scalar_mul(neg, combo[:, NCOL : NCOL + 1], -1.0)
            nc.scalar.activation(
                out=combo[:, :NCOL],
                in_=e_sb,
                func=mybir.ActivationFunctionType.Exp,
                bias=neg,
                scale=1.0,
                accum_out=combo[:, NCOL + 1 : NCOL + 2],
            )
            nc.sync.dma_start(out=out[:], in_=combo)
    nc.finalize()
    return nc


# Set by a driver (e.g. test.py) to capture a profiled run.
PROFILE = False
LAST_RESULT = None


def kernel(hidden, encoder_outputs, W, b):
    global _nc, LAST_RESULT
    from concourse.bass_utils import run_bass_kernel_spmd

    if _nc is None:
        _nc = _build()

    hidden = np.asarray(hidden)
    encoder_outputs = np.ascontiguousarray(np.asarray(encoder_outputs))
    W = np.asarray(W)

    u = (hidden.astype(np.float64) @ W.astype(np.float64)).astype(np.float32)

    in_maps = [
        {"enc": encoder_outputs[i * SSH : (i + 1) * SSH], "u": u}
        for i in range(NCORES)
    ]
    res = run_bass_kernel_spmd(
        _nc, in_maps, core_ids=list(range(NCORES)), trace=PROFILE
    )
    if PROFILE:
        LAST_RESULT = res

    outs = np.stack([r["out"] for r in res.results])  # [8, 128, 34]
    p_exp = outs[:, :, :NCOL].astype(np.float64)      # [8, 128, 32]
    m = outs[:, :, NCOL].astype(np.float64)           # [8, 128]
    ssum = outs[:, :, NCOL + 1].astype(np.float64)    # [8, 128]

    M = m.max()
    scale = np.exp(m - M)                             # [8, 128]
    Z = (ssum * scale).sum()
    attn = p_exp * (scale / Z)[:, :, None]            # [8, 128, 32]
    # element (core i, partition p, col c) is seq index i*SSH + c*P + p
    full = attn.transpose(0, 2, 1).reshape(-1).astype(np.float32)
    return full.reshape(1, 1, S)
